# revision 1
# baseline (speedup 1.0000x reference)
"""Causal attention (naive double-normalize reference == causal softmax) on 8 TRN2 cores.

Sharding:
  - Q rows interleaved: core i owns global rows {8l+i} -> uniform causal work per core.
  - K/V rows contiguous: core i computes rows [512i, 512(i+1)), AllGathers to all cores.

Per-core pipeline (matmuls bf16 with fp32 PSUM accumulation; K/V cross-core
payloads gathered in bf16):
  1. KT = Wk^T x_kv^T  [d, n_local] -> bounce -> AllGather (fires ~95us in).
  2. V  = x_kv Wv      [n_local, d] -> bounce -> AllGather (Wv prefetched, so
     V proj runs inside the AG_K window and AG_V fires right after AG_K).
  3. QT = Wq^T x_q^T   [d, m_local] (Wq streamed while the AGs run).
  4. scores^T tiles ST[n_tile, m] = KT_tile^T . QT; exp(scale*s) -> P (bf16),
     causal mask on the 16-column diagonal straddle, rowsum via ones-matmul.
  5. rowsum -> transpose (K=1 matmul) -> reciprocal.
  6. out[m, c] = sum_n P^T[n, m] V[n, c], scaled by reciprocal rowsum.

The math: reference does softmax -> tril mask -> renormalize; the unmasked
normalizer cancels exactly, leaving causal softmax. No max-subtraction needed:
scores/sqrt(d) ~ N(0,1), exp stays well within fp32 range.
"""

import math

import numpy as np

D = 2048          # d_in == d_out
CC = D // 128     # contraction chunks (16)
DT = D // 128     # output d tiles (16)
N_CORES = 8

_BUILT = {}


def _build(S):
    import concourse.bacc as bacc
    import concourse.mybir as mybir
    import concourse.tile as tile

    f32 = mybir.dt.float32
    bf16 = mybir.dt.bfloat16
    ML = S // N_CORES          # local q rows per core (512)
    NH = ML // 128             # output row tiles per core (4)
    NJ = S // 128              # key tiles (32)
    KTR = ML // 128            # key tiles per rank (4)
    SCALE = 1.0 / math.sqrt(D)
    EXP = mybir.ActivationFunctionType.Exp
    CPY = mybir.ActivationFunctionType.Copy
    RG = [list(range(N_CORES))]

    nc = bacc.Bacc("TRN2", target_bir_lowering=False)

    xq = nc.declare_dram_parameter("xq", [128, CC, ML], bf16, isOutput=False)
    xkv = nc.declare_dram_parameter("xkv", [128, CC, ML], bf16, isOutput=False)
    wq = nc.declare_dram_parameter("wq", [DT, 128, CC, 128], bf16, isOutput=False)
    wk = nc.declare_dram_parameter("wk", [DT, 128, CC, 128], bf16, isOutput=False)
    wv = nc.declare_dram_parameter("wv", [128, CC, D], bf16, isOutput=False)
    maskp = nc.declare_dram_parameter("mask", [128, 16], bf16, isOutput=False)
    out = nc.declare_dram_parameter("out", [ML, D], f32, isOutput=True)

    with tile.TileContext(nc) as tc:
        with (
            tc.tile_pool(name="const", bufs=1) as const,
            tc.tile_pool(name="dram", bufs=1, space="DRAM") as dram,
        ):
            qt_sb = const.tile([128, CC, ML], bf16)
            mask_sb = const.tile([128, 16], bf16)
            ones_sb = const.tile([128, 1], bf16)
            one1_sb = const.tile([1, 1], f32)
            rs_sb = const.tile([1, ML], f32)
            rin_sb = const.tile([128, NH], f32)
            recip_sb = const.tile([128, NH], f32)

            warm_sb = const.tile([1, 1], f32)
            nc.sync.dma_start(out=mask_sb[:], in_=maskp[:])
            nc.vector.memset(ones_sb[:], 1.0)
            nc.vector.memset(one1_sb[:], 1.0)
            # Load the Exp activation table while the PE warms up, instead of
            # lazily on the first score tile (critical path).
            nc.scalar.activation(
                out=warm_sb[:], in_=one1_sb[:],
                func=mybir.ActivationFunctionType.Exp,
            )

            kt_bounce = dram.tile([128, CC, ML], bf16)
            kt_ag = dram.tile([N_CORES * 128, CC, ML], bf16, addr_space="Shared")
            v_bounce = dram.tile([ML, D], bf16)
            v_ag = dram.tile([S, D], bf16, addr_space="Shared")

            # ============ projections ============
            with (
                tc.tile_pool(name="px", bufs=1) as px,
                tc.tile_pool(name="stage", bufs=4) as stage,
                tc.tile_pool(name="proj_ps", bufs=4, space="PSUM") as proj_ps,
            ):
                xkv_sb = px.tile([128, CC, ML], bf16)
                xq_sb = px.tile([128, CC, ML], bf16)
                nc.sync.dma_start(out=xkv_sb[:, 0:4, :], in_=xkv[:, 0:4, :])
                nc.sync.dma_start(out=xkv_sb[:, 4:CC, :], in_=xkv[:, 4:CC, :])

                # ---- KT projection -> bounce -> AG_K ----
                with (
                    tc.tile_pool(name="wkstream", bufs=4) as wkstream,
                    tc.tile_pool(name="wvhold", bufs=8) as wvhold,
                    tc.tile_pool(name="wqstream", bufs=8) as wqstream,
                    tc.spectator_scope("ktproj"),
                ):
                    for dt in range(DT):
                        w = wkstream.tile([128, CC, 128], bf16, tag="wk")
                        nc.sync.dma_start(out=w[:], in_=wk[dt])
                        ps = proj_ps.tile([128, ML], f32, tag="proj")
                        for c in range(CC):
                            nc.tensor.matmul(
                                out=ps[:], lhsT=w[:, c, :], rhs=xkv_sb[:, c, :],
                                start=(c == 0), stop=(c == CC - 1),
                            )
                        st = stage.tile([128, ML], bf16, tag="stage")
                        nc.vector.tensor_copy(out=st[:], in_=ps[:])
                        nc.sync.dma_start(out=kt_bounce[:, dt, :], in_=st[:])
                    nc.gpsimd.collective_compute(
                        "AllGather", mybir.AluOpType.bypass,
                        replica_groups=RG,
                        ins=[kt_bounce[:].opt()], outs=[kt_ag[:].opt()],
                    )

                    # ---- Wv prefetch + xq (land before AG_K hogs SDMA) ----
                    wv_tiles = []
                    for wc in range(8):
                        wvt = wvhold.tile(
                            [128, CC, 256], bf16, tag="wv", name=f"wv{wc}"
                        )
                        nc.sync.dma_start(
                            out=wvt[:], in_=wv[:, :, 256 * wc:256 * (wc + 1)]
                        )
                        wv_tiles.append(wvt)
                    nc.sync.dma_start(out=xq_sb[:], in_=xq[:])

                    # ---- Wq prefetch (behind Wv in the DMA queue) ----
                    wq_tiles = []
                    for dt in range(DT):
                        wqt = wqstream.tile(
                            [128, CC, 128], bf16, tag="wq", name=f"wq{dt}"
                        )
                        nc.sync.dma_start(out=wqt[:], in_=wq[dt])
                        wq_tiles.append(wqt)

                    def v_proj_half(cs_range):
                        for cs in cs_range:
                            for nt in range(ML // 128):
                                st = stage.tile(
                                    [128, ML], bf16, tag="stage",
                                    name=f"vst{cs}_{nt}",
                                )
                                for half in range(2):
                                    wvt = wv_tiles[2 * cs + half]
                                    ps = proj_ps.tile(
                                        [128, ML], f32, tag="proj",
                                        name=f"vps{cs}_{nt}_{half}",
                                    )
                                    for c in range(CC):
                                        nc.tensor.matmul(
                                            out=ps[:, 0:256],
                                            lhsT=xkv_sb[:, c, 128 * nt:128 * (nt + 1)],
                                            rhs=wvt[:, c, :],
                                            start=(c == 0), stop=(c == CC - 1),
                                        )
                                    nc.vector.tensor_copy(
                                        out=st[:, 256 * half:256 * (half + 1)],
                                        in_=ps[:, 0:256],
                                    )
                                nc.sync.dma_start(
                                    out=v_bounce[128 * nt:128 * (nt + 1),
                                                 512 * cs:512 * (cs + 1)],
                                    in_=st[:],
                                )

                    def qt_proj_half(dt_range):
                        for dt in dt_range:
                            ps = proj_ps.tile(
                                [128, ML], f32, tag="proj", name=f"qps{dt}"
                            )
                            for c in range(CC):
                                nc.tensor.matmul(
                                    out=ps[:], lhsT=wq_tiles[dt][:, c, :],
                                    rhs=xq_sb[:, c, :],
                                    start=(c == 0), stop=(c == CC - 1),
                                )
                            nc.vector.tensor_copy(out=qt_sb[:, dt, :], in_=ps[:])

                    # Interleave: V finishes early enough that AG_V queues right
                    # behind AG_K, while QT halves fill the remaining AG window
                    # so the score phase isn't gated on a late QT.
                    with tc.spectator_scope("vproj"):
                        v_proj_half(range(0, 2))
                    with tc.spectator_scope("qtproj_a"):
                        qt_proj_half(range(0, 8))
                    with tc.spectator_scope("vproj_b"):
                        v_proj_half(range(2, 4))
                        nc.gpsimd.collective_compute(
                            "AllGather", mybir.AluOpType.bypass,
                            replica_groups=RG,
                            ins=[v_bounce[:].opt()], outs=[v_ag[:].opt()],
                        )
                    with tc.spectator_scope("qtproj_b"):
                        qt_proj_half(range(8, DT))


            # ============ attention ============
            with (
                tc.tile_pool(name="attn", bufs=1) as attn,
                tc.tile_pool(name="ktstream", bufs=4) as ktstream,
                tc.tile_pool(name="vstream", bufs=6) as vstream,
                tc.tile_pool(name="avstage", bufs=8) as avstage,
                tc.tile_pool(name="outp", bufs=4) as outp,
                tc.tile_pool(name="st_ps", bufs=2, space="PSUM") as st_ps,
                tc.tile_pool(name="rs_ps", bufs=1, space="PSUM") as rs_ps,
                tc.tile_pool(name="av_ps", bufs=1, space="PSUM") as av_ps,
                tc.tile_pool(name="tp_ps", bufs=1, space="PSUM") as tp_ps,
            ):
                p_all = attn.tile([128, NJ, ML], bf16)
                rs = rs_ps.tile([1, ML], f32)
                with tc.spectator_scope("scores"):
                    for j in range(NJ):
                        r, n0 = j // KTR, 128 * (j % KTR)
                        if j % 2 == 0:
                            ktp = ktstream.tile(
                                [128, CC, 256], bf16, tag="kt", name=f"kt{j}"
                            )
                            nc.sync.dma_start(
                                out=ktp[:],
                                in_=kt_ag[128 * r:128 * (r + 1), :, n0:n0 + 256],
                            )
                            kt = ktp[:, :, 0:128]
                        else:
                            kt = ktp[:, :, 128:256]
                        m0 = 16 * j
                        ps = st_ps.tile([128, ML], f32, tag="st")
                        for c in range(CC):
                            nc.tensor.matmul(
                                out=ps[:, m0:ML], lhsT=kt[:, c, :],
                                rhs=qt_sb[:, c, m0:ML],
                                start=(c == 0), stop=(c == CC - 1),
                            )
                        pj = p_all[:, j, :]
                        nc.scalar.activation(
                            out=pj[:, m0:ML], in_=ps[:, m0:ML], func=EXP,
                            scale=SCALE,
                        )
                        nc.vector.tensor_tensor(
                            out=pj[:, m0:m0 + 16], in0=pj[:, m0:m0 + 16],
                            in1=mask_sb[:], op=mybir.AluOpType.mult,
                        )
                        g0 = 128 * (j // 8)
                        if m0 > g0:
                            nc.vector.memset(pj[:, g0:m0], 0.0)
                        nc.tensor.matmul(
                            out=rs[0:1, m0:ML], lhsT=ones_sb[:],
                            rhs=pj[:, m0:ML],
                            start=(j == 0), stop=(j == NJ - 1),
                        )

                with tc.spectator_scope("renorm"):
                    nc.vector.tensor_copy(out=rs_sb[:], in_=rs[:])
                    for h in range(NH):
                        tp = tp_ps.tile([128, 1], f32, tag="tp")
                        nc.tensor.matmul(
                            out=tp[:], lhsT=rs_sb[0:1, 128 * h:128 * (h + 1)],
                            rhs=one1_sb[:], start=True, stop=True,
                        )
                        nc.vector.tensor_copy(out=rin_sb[:, h:h + 1], in_=tp[:])
                    nc.vector.reciprocal(out=recip_sb[:], in_=rin_sb[:])

                with tc.spectator_scope("av"):
                    for cs in range(4):
                        av = [
                            av_ps.tile([128, 512], f32, tag=f"av{h}", name=f"av{h}_{cs}")
                            for h in range(NH)
                        ]
                        for t in range((NJ + 3) // 4):
                            vt = vstream.tile([128, 4, 512], bf16, tag="v")
                            nc.sync.dma_start(
                                out=vt[:],
                                in_=v_ag[512 * t:512 * (t + 1), 512 * cs:512 * (cs + 1)]
                                .rearrange("(jj p) n -> p jj n", p=128),
                            )
                            for jj in range(4):
                                j = 4 * t + jj
                                for h in range(j // 8, NH):
                                    nc.tensor.matmul(
                                        out=av[h][:],
                                        lhsT=p_all[:, j, 128 * h:128 * (h + 1)],
                                        rhs=vt[:, jj, :],
                                        start=(j == 0),
                                        stop=(j == min(8 * (h + 1), NJ) - 1),
                                    )
                        for h in range(NH):
                            # Unscaled copy frees the PSUM bank immediately so the
                            # next cs's accumulation never waits on the reciprocal
                            # (which waits on the whole score phase).
                            stg = avstage.tile(
                                [128, 512], f32, tag="avs", name=f"avs{h}_{cs}"
                            )
                            nc.vector.tensor_copy(out=stg[:], in_=av[h][:])
                            ob = outp.tile([128, 512], f32, tag="out")
                            nc.scalar.activation(
                                out=ob[:], in_=stg[:], func=CPY,
                                scale=recip_sb[:, h:h + 1],
                            )
                            nc.sync.dma_start(
                                out=out[128 * h:128 * (h + 1), 512 * cs:512 * (cs + 1)],
                                in_=ob[:],
                            )

    nc.finalize()
    return nc


def _prep_inputs(x, Wq, Wk, Wv, S):
    import ml_dtypes

    bf = ml_dtypes.bfloat16
    ML = S // N_CORES

    def shuf_w(W):
        # [dt, p, c, j] layout: element = W[128c+p, 128dt+j]
        return np.ascontiguousarray(
            W.reshape(CC, 128, DT, 128).transpose(2, 1, 0, 3)
        ).astype(bf)

    wq_h = shuf_w(Wq)
    wk_h = shuf_w(Wk)
    wv_h = np.ascontiguousarray(
        Wv.reshape(CC, 128, D).transpose(1, 0, 2)
    ).astype(bf)

    def shuf_x(rows):
        # rows [ML, D] -> [p, c, m] with element = rows[m, 128c+p]
        return np.ascontiguousarray(rows.reshape(ML, CC, 128).transpose(2, 1, 0)).astype(bf)

    in_maps = []
    for i in range(N_CORES):
        mask = (np.arange(128)[:, None] <= 8 * np.arange(16)[None, :] + i).astype(bf)
        in_maps.append({
            "xq": shuf_x(x[i::N_CORES]),
            "xkv": shuf_x(x[ML * i:ML * (i + 1)]),
            "wq": wq_h, "wk": wk_h, "wv": wv_h,
            "mask": mask,
        })
    return in_maps


def run(x, Wq, Wk, Wv, S, trace=False, trace_cores=None):
    from concourse.bass_utils import run_bass_kernel_spmd

    if S not in _BUILT:
        _BUILT[S] = _build(S)
    nc = _BUILT[S]
    in_maps = _prep_inputs(x, Wq, Wk, Wv, S)
    res = run_bass_kernel_spmd(
        nc, in_maps, list(range(N_CORES)), trace=trace, trace_cores=trace_cores
    )
    outs = [res.results[i]["out"] for i in range(N_CORES)]
    full = np.stack(outs, axis=1).reshape(S, D).astype(np.float32)
    return full, res


def kernel(x, Wq, Wk, Wv):
    x = np.asarray(x, dtype=np.float32)
    Wq = np.asarray(Wq, dtype=np.float32)
    Wk = np.asarray(Wk, dtype=np.float32)
    Wv = np.asarray(Wv, dtype=np.float32)
    full, _ = run(x, Wq, Wk, Wv, x.shape[0])
    return full

